# revision 59
# baseline (speedup 1.0000x reference)
"""Trainium2 Bass kernel for nn_Attention_51092930953251.

GQA attention with KV-cache at start_pos=1920 (total T=2048), B=8, S=128,
H=32, KVH=8, D=128. The harness-provided cache is all zeros, so positions
0..start_pos-1 contribute exactly exp(mask[s,t]) to the softmax denominator
and nothing to the numerator. The kernel computes attention over the 128
"live" positions; the cached region's denominator contribution is folded
into the additive mask as -log(sum_t<start exp(mask[s,t])) so the device
denominator is simply 1 + sum_live. Batch is sharded 1:1 across 8 cores.

Host-side input prep is pure layout work (batch sharding, transposes,
tiling the shared mask, appending a ones column to V) plus the mask-only
prefix constant; all q/k/v compute runs on device.

Self-contained: hardcodes all shapes; falls back to a numpy reference if
the inputs violate the assumptions (nonzero cache / different start_pos).
"""

import math

import numpy as np

B, S, DIM, KV_DIM = 8, 128, 4096, 1024
H, KVH, D = 32, 8, 128
NREP = H // KVH  # 4
START = 1920
T = START + S  # 2048
SCALE = 1.0 / math.sqrt(D)
NCORES = 8

# tuning flags
FP32R_S = False  # float32r for the scores matmul (unsupported by walrus)
BCAST_NORM = True  # batched normalize via step-0 broadcast AP
FP16_AV = False  # fp16 P and V for the AV matmul (single-pass PE)

_BUILT = {}


def _build_nc(fp32r_s=FP32R_S, bcast_norm=BCAST_NORM, fp16_av=FP16_AV):
    import concourse.bacc as bacc
    import concourse.mybir as mybir
    import concourse.tile as tile

    f32 = mybir.dt.float32
    f16 = mybir.dt.float16
    av_dt = f16 if fp16_av else f32
    AF = mybir.ActivationFunctionType
    ALU = mybir.AluOpType

    nc = bacc.Bacc(
        "TRN2", target_bir_lowering=False, debug=False, num_devices=NCORES
    )
    # group-blocked DRAM layouts: each chunk is a contiguous block.
    # q/k are fp16 hi|lo pairs per group (exact f32 split, same bytes).
    qt0_d = nc.dram_tensor("qT0", [4, 128, S], f32, kind="ExternalInput")
    qt_d = nc.dram_tensor(
        "qT", [KVH - 1, 128, NREP * S], f32, kind="ExternalInput"
    )
    kt_d = nc.dram_tensor("kT", [KVH, 128, S], f32, kind="ExternalInput")
    v_d = nc.dram_tensor(
        "vones", [2, S, 4 * (D + 1)], av_dt, kind="ExternalInput"
    )
    mt4_d = nc.dram_tensor("maskT4", [S, NREP * S], f32, kind="ExternalInput")
    out_d = nc.dram_tensor("out", [KVH, S, NREP * D], f32, kind="ExternalOutput")

    with tile.TileContext(nc) as tc:
        with (
            tc.tile_pool(name="big", bufs=1) as big,
            tc.tile_pool(name="work", bufs=3) as work,
            tc.tile_pool(name="small", bufs=6) as small,
            tc.tile_pool(name="og", bufs=3) as ogp,
            tc.tile_pool(name="ps_s", bufs=3, space="PSUM") as ps_s,
            tc.tile_pool(name="ps_o", bufs=4, space="PSUM") as ps_o,
            tc.tile_pool(name="ps_w", bufs=1, space="PSUM") as ps_w,
        ):
            qt_sb = big.tile([128, H * S], f32, tag="qT")
            kt_sb = big.tile([128, KVH * S], f32, tag="kT")
            v_sb = big.tile([S, KVH * (D + 1)], av_dt, tag="v")
            mt4_sb = big.tile([S, NREP * S], f32, tag="mt4")

            def load_k(g, eng):
                eng.dma_start(
                    kt_sb[:, g * 128 : (g + 1) * 128], kt_d.ap()[g]
                )

            def load_q(g):
                nc.sync.dma_start(
                    qt_sb[:, g * 512 : (g + 1) * 512], qt_d.ap()[g - 1]
                )

            # DMA completions drain in global dispatch-time order, so ALL
            # loads go on one queue in exact need-order; only group 0's q
            # is split into 64KB chunks so the first matmul starts early
            load_k(0, nc.sync)
            for c in range(4):
                nc.sync.dma_start(
                    qt_sb[:, c * 128 : (c + 1) * 128], qt0_d.ap()[c]
                )
            load_k(1, nc.sync)
            load_q(1)
            nc.sync.dma_start(mt4_sb[:, :], mt4_d.ap())
            nc.sync.dma_start(v_sb[:, 0:258], v_d.ap()[0][:, 0:258])
            load_k(2, nc.sync)
            load_q(2)
            load_k(3, nc.sync)
            load_q(3)
            nc.sync.dma_start(v_sb[:, 258:516], v_d.ap()[0][:, 258:516])
            load_k(4, nc.sync)
            load_q(4)
            load_k(5, nc.sync)
            load_q(5)
            nc.sync.dma_start(v_sb[:, 516:1032], v_d.ap()[1])
            load_k(6, nc.sync)
            load_q(6)
            load_k(7, nc.sync)
            load_q(7)

            # warm the PE (HAM clock gate: first ~3.4us of activity runs
            # at 1.2GHz) with throwaway fp16 matmuls while loads land
            warm_sb = big.tile([128, 128], f16, tag="warm")
            nc.gpsimd.memset(warm_sb[:, :], 0.0)
            warm_ps = ps_w.tile([128, 128], f32, tag="warm_ps")

            def emit_warm(n):
                for _ in range(n):
                    nc.tensor.matmul(
                        warm_ps[:, :], warm_sb[:, :], warm_sb[:, :]
                    )

            emit_warm(14)

            def emit_s(g):
                # S^T = K_g @ Q_g^T : [t', 4s]
                sT_ps = ps_s.tile([128, NREP * 128], f32, tag="sT")
                nc.tensor.matmul(
                    sT_ps[:, :],
                    kt_sb[:, g * 128 : (g + 1) * 128],
                    qt_sb[:, g * 512 : (g + 1) * 512],
                )
                return sT_ps

            def emit_softmax(g, sT_ps):
                # scaled scores + mask (mask has -log(presum) folded in)
                spre_sb = work.tile([128, NREP * 128], f32, tag="spre")
                nc.vector.scalar_tensor_tensor(
                    spre_sb[:, :], sT_ps[:, :], SCALE, mt4_sb[:, :],
                    ALU.mult, ALU.add,
                )
                pT_sb = work.tile([128, NREP * 128], av_dt, tag="pT")
                nc.scalar.activation(pT_sb[:, :], spre_sb[:, :], AF.Exp)
                return pT_sb

            def emit_av(g, pT_sb):
                # AV with ones column, two heads packed per PSUM tile
                o_tiles = []
                for j in range(2):
                    o_ps = ps_o.tile([128, 2 * (D + 1)], f32, tag="o")
                    o_tiles.append(o_ps)
                    for i in range(2):
                        r = 2 * j + i
                        nc.tensor.matmul(
                            o_ps[:, i * (D + 1) : (i + 1) * (D + 1)],
                            pT_sb[:, r * 128 : (r + 1) * 128],
                            v_sb[:, g * (D + 1) : (g + 1) * (D + 1)],
                        )
                return o_tiles

            def emit_denoms(g, o_tiles):
                recips = []
                for j in range(2):
                    o_r = o_tiles[j][:, :].rearrange("p (c x) -> p c x", c=2)
                    denom = small.tile([128, 2], f32, tag="denom")
                    recip = small.tile([128, 2], f32, tag="recip")
                    # denom = rowsum + 1  (the +1 is the normalized prefix)
                    nc.vector.tensor_scalar_add(denom[:, :], o_r[:, :, D], 1.0)
                    nc.vector.reciprocal(recip[:, :], denom[:, :])
                    recips.append(recip)
                return recips

            def emit_norms(g, o_tiles, recips):
                og_sb = ogp.tile([128, NREP * 128], f32, tag="og")
                for j in range(2):
                    o_r = o_tiles[j][:, :].rearrange("p (c x) -> p c x", c=2)
                    recip = recips[j]
                    if bcast_norm and j == 0:
                        nc.vector.tensor_tensor(
                            og_sb[:, j * 256 : (j + 1) * 256].rearrange(
                                "p (c x) -> p c x", c=2
                            ),
                            o_r[:, :, 0:D],
                            recip[:, :].broadcast_to([128, 2, D]),
                            ALU.mult,
                        )
                    else:
                        # normalize on the scalar engine (Copy shares the
                        # Exp table slot, no reload)
                        for i in range(2):
                            r = 2 * j + i
                            nc.scalar.activation(
                                og_sb[:, r * 128 : (r + 1) * 128],
                                o_r[:, i, 0:D],
                                AF.Copy,
                                scale=recip[:, i : i + 1],
                            )
                if g >= 7:
                    # split the last store so its final bytes land sooner
                    for c in range(4):
                        eng = nc.sync if c % 2 == 0 else nc.scalar
                        eng.dma_start(
                            out_d.ap()[g][:, c * 128 : (c + 1) * 128],
                            og_sb[:, c * 128 : (c + 1) * 128],
                        )
                else:
                    nc.sync.dma_start(out_d.ap()[g], og_sb[:, :])

            # software pipeline: S runs 3 groups ahead; next group's
            # stt/exp is issued before this group's norm copies so the
            # scalar queue never blocks the exp chain
            # dummy matmuls between the early real ones keep the PE's
            # HAM activity window busy while loads land, so the first
            # groups run at 2.4GHz instead of re-throttled 1.2GHz
            sT = {0: emit_s(0)}
            emit_warm(6)
            sT[1] = emit_s(1)
            emit_warm(4)
            pT = {0: emit_softmax(0, sT.pop(0))}
            sT[2] = emit_s(2)
            emit_warm(2)
            prev = None  # (g, o_tiles, recips)
            for g in range(KVH):
                o_tiles = emit_av(g, pT.pop(g))
                recips = emit_denoms(g, o_tiles)
                if g + 1 < KVH:
                    pT[g + 1] = emit_softmax(g + 1, sT.pop(g + 1))
                if g + 3 < KVH:
                    sT[g + 3] = emit_s(g + 3)
                if prev is not None:
                    emit_norms(*prev)
                prev = (g, o_tiles, recips)
            emit_norms(*prev)

    nc.compile()
    return nc


def _get_nc(**kw):
    key = tuple(sorted(kw.items()))
    if key not in _BUILT:
        _BUILT[key] = _build_nc(**kw)
    return _BUILT[key]


def _reference_fallback(q, k, v, start_pos, mask, cache_k, cache_v):
    b, s, _ = q.shape
    start_pos = int(start_pos)
    t = start_pos + s
    xq = q.reshape(b, s, H, D).astype(np.float32)
    xk = k.reshape(b, s, KVH, D).astype(np.float32)
    xv = v.reshape(b, s, KVH, D).astype(np.float32)
    ck = np.array(cache_k[:b, :t], dtype=np.float32, copy=True)
    cv = np.array(cache_v[:b, :t], dtype=np.float32, copy=True)
    ck[:, start_pos:t] = xk
    cv[:, start_pos:t] = xv
    xqg = xq.reshape(b, s, KVH, NREP, D)
    scores = np.einsum("bsgrd,btgd->bgrst", xqg, ck) * SCALE
    scores = scores + np.asarray(mask, dtype=np.float32)[:, :, None]
    scores -= scores.max(axis=-1, keepdims=True)
    p = np.exp(scores)
    p /= p.sum(axis=-1, keepdims=True)
    out = np.einsum("bgrst,btgd->bsgrd", p, cv)
    return out.reshape(b, s, H * D).astype(np.float32)


def kernel(q, k, v, start_pos, freqs_cis, mask, cache_k, cache_v):
    q = np.asarray(q, dtype=np.float32)
    k = np.asarray(k, dtype=np.float32)
    v = np.asarray(v, dtype=np.float32)
    mask = np.asarray(mask, dtype=np.float32)
    sp = int(start_pos)

    fast_ok = (
        sp == START
        and q.shape == (B, S, DIM)
        and k.shape == (B, S, KV_DIM)
        and v.shape == (B, S, KV_DIM)
        and mask.shape == (1, 1, S, T)
        and not np.asarray(cache_k)[:B, :START].any()
        and not np.asarray(cache_v)[:B, :START].any()
    )
    if not fast_ok:
        return _reference_fallback(q, k, v, sp, mask, cache_k, cache_v)

    from concourse.bass_utils import run_bass_kernel_spmd

    nc = _get_nc(fp32r_s=FP32R_S, bcast_norm=BCAST_NORM, fp16_av=FP16_AV)

    m2d = mask[0, 0]  # [S, T]
    presum = np.exp(m2d[:, :START]).sum(axis=1)  # [S]
    mlive_t = m2d[:, START:].T - np.log(presum)[None, :]  # [t', s]
    mask_t4 = np.ascontiguousarray(np.tile(mlive_t, (1, NREP)), np.float32)

    # host layout prep (pure permutation): group-blocked transposes,
    # ones column for V
    # qT[b, g, d, r*S+s] = q[b, s, (g*NREP+r)*D + d], as fp16 hi|lo pair
    qt32 = q.reshape(B, S, KVH, NREP, D).transpose(0, 2, 4, 3, 1).reshape(
        B, KVH, 128, NREP * S
    )
    qt = np.ascontiguousarray(qt32, np.float32)
    # group 0 additionally as 4 contiguous 64KB chunks
    qt0 = np.ascontiguousarray(
        qt[:, 0].reshape(B, 128, 4, S).transpose(0, 2, 1, 3)
    )
    qt = np.ascontiguousarray(qt[:, 1:])
    # kT[b, g, d, t'] = k[b, t', g*D + d]
    kt = np.ascontiguousarray(
        k.reshape(B, S, KVH, D).transpose(0, 2, 3, 1), np.float32
    )
    v_dt = np.float16 if FP16_AV else np.float32
    vones = np.empty((B, S, KVH, D + 1), dtype=v_dt)
    vones[..., :D] = v.reshape(B, S, KVH, D)
    vones[..., D] = 1.0
    vones = np.ascontiguousarray(
        vones.reshape(B, S, 2, 4 * (D + 1)).transpose(0, 2, 1, 3)
    )

    in_maps = [
        {
            "qT0": qt0[b],
            "qT": qt[b],
            "kT": kt[b],
            "vones": vones[b],
            "maskT4": mask_t4,
        }
        for b in range(B)
    ]
    res = run_bass_kernel_spmd(nc, in_maps, list(range(NCORES)))
    # device out is [KVH, S, NREP*D] blocks; un-permute to [S, H*D]
    out = np.stack(
        [
            res.results[b]["out"].transpose(1, 0, 2).reshape(S, DIM)
            for b in range(B)
        ],
        axis=0,
    )
    return out


# revision 60
# speedup vs baseline: 1.0098x; 1.0098x over previous
"""Trainium2 Bass kernel for nn_Attention_51092930953251.

GQA attention with KV-cache at start_pos=1920 (total T=2048), B=8, S=128,
H=32, KVH=8, D=128. The harness-provided cache is all zeros, so positions
0..start_pos-1 contribute exactly exp(mask[s,t]) to the softmax denominator
and nothing to the numerator. The kernel computes attention over the 128
"live" positions; the cached region's denominator contribution is folded
into the additive mask as -log(sum_t<start exp(mask[s,t])) so the device
denominator is simply 1 + sum_live. Batch is sharded 1:1 across 8 cores.

Host-side input prep is pure layout work (batch sharding, transposes,
tiling the shared mask, appending a ones column to V) plus the mask-only
prefix constant; all q/k/v compute runs on device.

Self-contained: hardcodes all shapes; falls back to a numpy reference if
the inputs violate the assumptions (nonzero cache / different start_pos).
"""

import math

import numpy as np

B, S, DIM, KV_DIM = 8, 128, 4096, 1024
H, KVH, D = 32, 8, 128
NREP = H // KVH  # 4
START = 1920
T = START + S  # 2048
SCALE = 1.0 / math.sqrt(D)
NCORES = 8

# tuning flags
FP32R_S = False  # float32r for the scores matmul (unsupported by walrus)
BCAST_NORM = True  # batched normalize via step-0 broadcast AP
FP16_AV = False  # fp16 P and V for the AV matmul (single-pass PE)

_BUILT = {}


def _build_nc(fp32r_s=FP32R_S, bcast_norm=BCAST_NORM, fp16_av=FP16_AV):
    import concourse.bacc as bacc
    import concourse.mybir as mybir
    import concourse.tile as tile

    f32 = mybir.dt.float32
    f16 = mybir.dt.float16
    av_dt = f16 if fp16_av else f32
    AF = mybir.ActivationFunctionType
    ALU = mybir.AluOpType

    nc = bacc.Bacc(
        "TRN2", target_bir_lowering=False, debug=False, num_devices=NCORES
    )
    # group-blocked DRAM layouts: each chunk is a contiguous block.
    # q/k are fp16 hi|lo pairs per group (exact f32 split, same bytes).
    qt0_d = nc.dram_tensor("qT0", [4, 128, S], f32, kind="ExternalInput")
    qt_d = nc.dram_tensor(
        "qT", [KVH - 1, 128, NREP * S], f32, kind="ExternalInput"
    )
    kt_d = nc.dram_tensor("kT", [KVH, 128, S], f32, kind="ExternalInput")
    v_d = nc.dram_tensor(
        "vones", [2, S, 4 * (D + 1)], av_dt, kind="ExternalInput"
    )
    mt4_d = nc.dram_tensor("maskT4", [S, NREP * S], f32, kind="ExternalInput")
    out_d = nc.dram_tensor("out", [KVH, S, NREP * D], f32, kind="ExternalOutput")

    with tile.TileContext(nc) as tc:
        with (
            tc.tile_pool(name="big", bufs=1) as big,
            tc.tile_pool(name="work", bufs=3) as work,
            tc.tile_pool(name="small", bufs=6) as small,
            tc.tile_pool(name="og", bufs=3) as ogp,
            tc.tile_pool(name="ps_s", bufs=3, space="PSUM") as ps_s,
            tc.tile_pool(name="ps_o", bufs=4, space="PSUM") as ps_o,
            tc.tile_pool(name="ps_w", bufs=1, space="PSUM") as ps_w,
        ):
            qt_sb = big.tile([128, H * S], f32, tag="qT")
            kt_sb = big.tile([128, KVH * S], f32, tag="kT")
            v_sb = big.tile([S, KVH * (D + 1)], av_dt, tag="v")
            mt4_sb = big.tile([S, NREP * S], f32, tag="mt4")

            def load_k(g, eng):
                eng.dma_start(
                    kt_sb[:, g * 128 : (g + 1) * 128], kt_d.ap()[g]
                )

            def load_q(g):
                nc.sync.dma_start(
                    qt_sb[:, g * 512 : (g + 1) * 512], qt_d.ap()[g - 1]
                )

            # DMA completions drain in global dispatch-time order, so ALL
            # loads go on one queue in exact need-order; only group 0's q
            # is split into 64KB chunks so the first matmul starts early
            load_k(0, nc.sync)
            for c in range(4):
                nc.sync.dma_start(
                    qt_sb[:, c * 128 : (c + 1) * 128], qt0_d.ap()[c]
                )
            load_k(1, nc.sync)
            load_q(1)
            nc.sync.dma_start(mt4_sb[:, :], mt4_d.ap())
            nc.sync.dma_start(v_sb[:, 0:258], v_d.ap()[0][:, 0:258])
            load_k(2, nc.sync)
            load_q(2)
            load_k(3, nc.sync)
            load_q(3)
            nc.sync.dma_start(v_sb[:, 258:516], v_d.ap()[0][:, 258:516])
            load_k(4, nc.sync)
            load_q(4)
            load_k(5, nc.sync)
            load_q(5)
            nc.sync.dma_start(v_sb[:, 516:1032], v_d.ap()[1])
            load_k(6, nc.sync)
            load_q(6)
            load_k(7, nc.sync)
            load_q(7)

            # warm the PE (HAM clock gate: first ~3.4us of activity runs
            # at 1.2GHz) with throwaway fp16 matmuls while loads land
            warm_sb = big.tile([128, 128], f16, tag="warm")
            nc.gpsimd.memset(warm_sb[:, :], 0.0)
            warm_ps = ps_w.tile([128, 128], f32, tag="warm_ps")
            for _ in range(14):
                nc.tensor.matmul(warm_ps[:, :], warm_sb[:, :], warm_sb[:, :])

            def emit_s(g):
                # S^T = K_g @ Q_g^T : [t', 4s]
                sT_ps = ps_s.tile([128, NREP * 128], f32, tag="sT")
                nc.tensor.matmul(
                    sT_ps[:, :],
                    kt_sb[:, g * 128 : (g + 1) * 128],
                    qt_sb[:, g * 512 : (g + 1) * 512],
                )
                return sT_ps

            def emit_softmax(g, sT_ps):
                # scaled scores + mask (mask has -log(presum) folded in)
                spre_sb = work.tile([128, NREP * 128], f32, tag="spre")
                nc.vector.scalar_tensor_tensor(
                    spre_sb[:, :], sT_ps[:, :], SCALE, mt4_sb[:, :],
                    ALU.mult, ALU.add,
                )
                pT_sb = work.tile([128, NREP * 128], av_dt, tag="pT")
                nc.scalar.activation(pT_sb[:, :], spre_sb[:, :], AF.Exp)
                return pT_sb

            def emit_av(g, pT_sb):
                # AV with ones column, two heads packed per PSUM tile
                o_tiles = []
                for j in range(2):
                    o_ps = ps_o.tile([128, 2 * (D + 1)], f32, tag="o")
                    o_tiles.append(o_ps)
                    for i in range(2):
                        r = 2 * j + i
                        nc.tensor.matmul(
                            o_ps[:, i * (D + 1) : (i + 1) * (D + 1)],
                            pT_sb[:, r * 128 : (r + 1) * 128],
                            v_sb[:, g * (D + 1) : (g + 1) * (D + 1)],
                        )
                return o_tiles

            def emit_denoms(g, o_tiles):
                recips = []
                for j in range(2):
                    o_r = o_tiles[j][:, :].rearrange("p (c x) -> p c x", c=2)
                    denom = small.tile([128, 2], f32, tag="denom")
                    recip = small.tile([128, 2], f32, tag="recip")
                    # denom = rowsum + 1  (the +1 is the normalized prefix)
                    nc.vector.tensor_scalar_add(denom[:, :], o_r[:, :, D], 1.0)
                    nc.vector.reciprocal(recip[:, :], denom[:, :])
                    recips.append(recip)
                return recips

            def emit_norms(g, o_tiles, recips):
                og_sb = ogp.tile([128, NREP * 128], f32, tag="og")
                for j in range(2):
                    o_r = o_tiles[j][:, :].rearrange("p (c x) -> p c x", c=2)
                    recip = recips[j]
                    if bcast_norm and j == 0:
                        nc.vector.tensor_tensor(
                            og_sb[:, j * 256 : (j + 1) * 256].rearrange(
                                "p (c x) -> p c x", c=2
                            ),
                            o_r[:, :, 0:D],
                            recip[:, :].broadcast_to([128, 2, D]),
                            ALU.mult,
                        )
                    else:
                        # normalize on the scalar engine (Copy shares the
                        # Exp table slot, no reload)
                        for i in range(2):
                            r = 2 * j + i
                            nc.scalar.activation(
                                og_sb[:, r * 128 : (r + 1) * 128],
                                o_r[:, i, 0:D],
                                AF.Copy,
                                scale=recip[:, i : i + 1],
                            )
                if g >= 7:
                    # split the last store so its final bytes land sooner
                    for c in range(4):
                        eng = nc.sync if c % 2 == 0 else nc.scalar
                        eng.dma_start(
                            out_d.ap()[g][:, c * 128 : (c + 1) * 128],
                            og_sb[:, c * 128 : (c + 1) * 128],
                        )
                else:
                    nc.sync.dma_start(out_d.ap()[g], og_sb[:, :])

            # software pipeline: S runs 3 groups ahead; next group's
            # stt/exp is issued before this group's norm copies so the
            # scalar queue never blocks the exp chain
            sT = {0: emit_s(0), 1: emit_s(1)}
            pT = {0: emit_softmax(0, sT.pop(0))}
            sT[2] = emit_s(2)
            prev = None  # (g, o_tiles, recips)
            for g in range(KVH):
                o_tiles = emit_av(g, pT.pop(g))
                recips = emit_denoms(g, o_tiles)
                if g + 1 < KVH:
                    pT[g + 1] = emit_softmax(g + 1, sT.pop(g + 1))
                if g + 3 < KVH:
                    sT[g + 3] = emit_s(g + 3)
                if prev is not None:
                    emit_norms(*prev)
                prev = (g, o_tiles, recips)
            emit_norms(*prev)

    nc.compile()
    return nc


def _get_nc(**kw):
    key = tuple(sorted(kw.items()))
    if key not in _BUILT:
        _BUILT[key] = _build_nc(**kw)
    return _BUILT[key]


def _reference_fallback(q, k, v, start_pos, mask, cache_k, cache_v):
    b, s, _ = q.shape
    start_pos = int(start_pos)
    t = start_pos + s
    xq = q.reshape(b, s, H, D).astype(np.float32)
    xk = k.reshape(b, s, KVH, D).astype(np.float32)
    xv = v.reshape(b, s, KVH, D).astype(np.float32)
    ck = np.array(cache_k[:b, :t], dtype=np.float32, copy=True)
    cv = np.array(cache_v[:b, :t], dtype=np.float32, copy=True)
    ck[:, start_pos:t] = xk
    cv[:, start_pos:t] = xv
    xqg = xq.reshape(b, s, KVH, NREP, D)
    scores = np.einsum("bsgrd,btgd->bgrst", xqg, ck) * SCALE
    scores = scores + np.asarray(mask, dtype=np.float32)[:, :, None]
    scores -= scores.max(axis=-1, keepdims=True)
    p = np.exp(scores)
    p /= p.sum(axis=-1, keepdims=True)
    out = np.einsum("bgrst,btgd->bsgrd", p, cv)
    return out.reshape(b, s, H * D).astype(np.float32)


def kernel(q, k, v, start_pos, freqs_cis, mask, cache_k, cache_v):
    q = np.asarray(q, dtype=np.float32)
    k = np.asarray(k, dtype=np.float32)
    v = np.asarray(v, dtype=np.float32)
    mask = np.asarray(mask, dtype=np.float32)
    sp = int(start_pos)

    fast_ok = (
        sp == START
        and q.shape == (B, S, DIM)
        and k.shape == (B, S, KV_DIM)
        and v.shape == (B, S, KV_DIM)
        and mask.shape == (1, 1, S, T)
        and not np.asarray(cache_k)[:B, :START].any()
        and not np.asarray(cache_v)[:B, :START].any()
    )
    if not fast_ok:
        return _reference_fallback(q, k, v, sp, mask, cache_k, cache_v)

    from concourse.bass_utils import run_bass_kernel_spmd

    nc = _get_nc(fp32r_s=FP32R_S, bcast_norm=BCAST_NORM, fp16_av=FP16_AV)

    m2d = mask[0, 0]  # [S, T]
    presum = np.exp(m2d[:, :START]).sum(axis=1)  # [S]
    mlive_t = m2d[:, START:].T - np.log(presum)[None, :]  # [t', s]
    mask_t4 = np.ascontiguousarray(np.tile(mlive_t, (1, NREP)), np.float32)

    # host layout prep (pure permutation): group-blocked transposes,
    # ones column for V
    # qT[b, g, d, r*S+s] = q[b, s, (g*NREP+r)*D + d], as fp16 hi|lo pair
    qt32 = q.reshape(B, S, KVH, NREP, D).transpose(0, 2, 4, 3, 1).reshape(
        B, KVH, 128, NREP * S
    )
    qt = np.ascontiguousarray(qt32, np.float32)
    # group 0 additionally as 4 contiguous 64KB chunks
    qt0 = np.ascontiguousarray(
        qt[:, 0].reshape(B, 128, 4, S).transpose(0, 2, 1, 3)
    )
    qt = np.ascontiguousarray(qt[:, 1:])
    # kT[b, g, d, t'] = k[b, t', g*D + d]
    kt = np.ascontiguousarray(
        k.reshape(B, S, KVH, D).transpose(0, 2, 3, 1), np.float32
    )
    v_dt = np.float16 if FP16_AV else np.float32
    vones = np.empty((B, S, KVH, D + 1), dtype=v_dt)
    vones[..., :D] = v.reshape(B, S, KVH, D)
    vones[..., D] = 1.0
    vones = np.ascontiguousarray(
        vones.reshape(B, S, 2, 4 * (D + 1)).transpose(0, 2, 1, 3)
    )

    in_maps = [
        {
            "qT0": qt0[b],
            "qT": qt[b],
            "kT": kt[b],
            "vones": vones[b],
            "maskT4": mask_t4,
        }
        for b in range(B)
    ]
    res = run_bass_kernel_spmd(nc, in_maps, list(range(NCORES)))
    # device out is [KVH, S, NREP*D] blocks; un-permute to [S, H*D]
    out = np.stack(
        [
            res.results[b]["out"].transpose(1, 0, 2).reshape(S, DIM)
            for b in range(B)
        ],
        axis=0,
    )
    return out


# revision 63
# speedup vs baseline: 1.0292x; 1.0193x over previous
"""Trainium2 Bass kernel for nn_Attention_51092930953251.

GQA attention with KV-cache at start_pos=1920 (total T=2048), B=8, S=128,
H=32, KVH=8, D=128. The harness-provided cache is all zeros, so positions
0..start_pos-1 contribute exactly exp(mask[s,t]) to the softmax denominator
and nothing to the numerator. The kernel computes attention over the 128
"live" positions; the cached region's denominator contribution is folded
into the additive mask as -log(sum_t<start exp(mask[s,t])) so the device
denominator is simply 1 + sum_live. Batch is sharded 1:1 across 8 cores.

Host-side input prep is pure layout work (batch sharding, transposes,
tiling the shared mask, appending a ones column to V) plus the mask-only
prefix constant; all q/k/v compute runs on device.

Self-contained: hardcodes all shapes; falls back to a numpy reference if
the inputs violate the assumptions (nonzero cache / different start_pos).
"""

import math

import numpy as np

B, S, DIM, KV_DIM = 8, 128, 4096, 1024
H, KVH, D = 32, 8, 128
NREP = H // KVH  # 4
START = 1920
T = START + S  # 2048
SCALE = 1.0 / math.sqrt(D)
NCORES = 8

# tuning flags
FP32R_S = False  # float32r for the scores matmul (unsupported by walrus)
BCAST_NORM = True  # batched normalize via step-0 broadcast AP
FP16_AV = False  # fp16 P and V for the AV matmul (single-pass PE)

_BUILT = {}


def _build_nc(fp32r_s=FP32R_S, bcast_norm=BCAST_NORM, fp16_av=FP16_AV):
    import concourse.bacc as bacc
    import concourse.mybir as mybir
    import concourse.tile as tile

    f32 = mybir.dt.float32
    f16 = mybir.dt.float16
    av_dt = f16 if fp16_av else f32
    AF = mybir.ActivationFunctionType
    ALU = mybir.AluOpType

    nc = bacc.Bacc(
        "TRN2", target_bir_lowering=False, debug=False, num_devices=NCORES
    )
    # group-blocked DRAM layouts: each chunk is a contiguous block.
    # q/k are fp16 hi|lo pairs per group (exact f32 split, same bytes).
    qt0_d = nc.dram_tensor("qT0", [4, 128, S], f32, kind="ExternalInput")
    qt_d = nc.dram_tensor(
        "qT", [KVH - 1, 128, NREP * S], f32, kind="ExternalInput"
    )
    kt_d = nc.dram_tensor("kT", [KVH, 128, S], f32, kind="ExternalInput")
    v_d = nc.dram_tensor(
        "vones", [2, S, 4 * (D + 1)], av_dt, kind="ExternalInput"
    )
    mt4_d = nc.dram_tensor("maskT4", [S, NREP * S], f32, kind="ExternalInput")
    out_d = nc.dram_tensor("out", [KVH, S, NREP * D], f32, kind="ExternalOutput")

    with tile.TileContext(nc) as tc:
        with (
            tc.tile_pool(name="big", bufs=1) as big,
            tc.tile_pool(name="work", bufs=3) as work,
            tc.tile_pool(name="small", bufs=6) as small,
            tc.tile_pool(name="og", bufs=3) as ogp,
            tc.tile_pool(name="ps_s", bufs=3, space="PSUM") as ps_s,
            tc.tile_pool(name="ps_o", bufs=5, space="PSUM") as ps_o,
        ):
            qt_sb = big.tile([128, H * S], f32, tag="qT")
            kt_sb = big.tile([128, KVH * S], f32, tag="kT")
            v_sb = big.tile([S, KVH * (D + 1)], av_dt, tag="v")
            mt4_sb = big.tile([S, NREP * S], f32, tag="mt4")

            def load_k(g, eng):
                eng.dma_start(
                    kt_sb[:, g * 128 : (g + 1) * 128], kt_d.ap()[g]
                )

            def load_q(g):
                nc.sync.dma_start(
                    qt_sb[:, g * 512 : (g + 1) * 512], qt_d.ap()[g - 1]
                )

            # DMA completions drain in global dispatch-time order, so ALL
            # loads go on one queue in exact need-order; only group 0's q
            # is split into 64KB chunks so the first matmul starts early
            load_k(0, nc.sync)
            for c in range(4):
                nc.sync.dma_start(
                    qt_sb[:, c * 128 : (c + 1) * 128], qt0_d.ap()[c]
                )
            load_k(1, nc.sync)
            load_q(1)
            nc.sync.dma_start(mt4_sb[:, :], mt4_d.ap())
            nc.sync.dma_start(v_sb[:, 0:258], v_d.ap()[0][:, 0:258])
            load_k(2, nc.sync)
            load_q(2)
            load_k(3, nc.sync)
            load_q(3)
            nc.sync.dma_start(v_sb[:, 258:516], v_d.ap()[0][:, 258:516])
            load_k(4, nc.sync)
            load_q(4)
            load_k(5, nc.sync)
            load_q(5)
            nc.sync.dma_start(v_sb[:, 516:1032], v_d.ap()[1])
            load_k(6, nc.sync)
            load_q(6)
            load_k(7, nc.sync)
            load_q(7)

            # warm the PE (HAM clock gate: first ~3.4us of activity runs
            # at 1.2GHz) with throwaway fp16 matmuls while loads land
            warm_sb = big.tile([128, 128], f16, tag="warm")
            nc.gpsimd.memset(warm_sb[:, :], 0.0)
            warm_ps = ps_s.tile([128, NREP * 128], f32, tag="sT")
            for _ in range(14):
                nc.tensor.matmul(
                    warm_ps[:, 0:128], warm_sb[:, :], warm_sb[:, :]
                )

            def emit_s(g):
                # S^T = K_g @ Q_g^T : [t', 4s]
                sT_ps = ps_s.tile([128, NREP * 128], f32, tag="sT")
                nc.tensor.matmul(
                    sT_ps[:, :],
                    kt_sb[:, g * 128 : (g + 1) * 128],
                    qt_sb[:, g * 512 : (g + 1) * 512],
                )
                return sT_ps

            def emit_softmax(g, sT_ps):
                # scaled scores + mask (mask has -log(presum) folded in)
                spre_sb = work.tile([128, NREP * 128], f32, tag="spre")
                nc.vector.scalar_tensor_tensor(
                    spre_sb[:, :], sT_ps[:, :], SCALE, mt4_sb[:, :],
                    ALU.mult, ALU.add,
                )
                pT_sb = work.tile([128, NREP * 128], av_dt, tag="pT")
                nc.scalar.activation(pT_sb[:, :], spre_sb[:, :], AF.Exp)
                return pT_sb

            def emit_av(g, pT_sb):
                # AV with ones column, two heads packed per PSUM tile
                o_tiles = []
                for j in range(2):
                    o_ps = ps_o.tile([128, 2 * (D + 1)], f32, tag="o")
                    o_tiles.append(o_ps)
                    for i in range(2):
                        r = 2 * j + i
                        nc.tensor.matmul(
                            o_ps[:, i * (D + 1) : (i + 1) * (D + 1)],
                            pT_sb[:, r * 128 : (r + 1) * 128],
                            v_sb[:, g * (D + 1) : (g + 1) * (D + 1)],
                        )
                return o_tiles

            def emit_denoms(g, o_tiles):
                recips = []
                for j in range(2):
                    o_r = o_tiles[j][:, :].rearrange("p (c x) -> p c x", c=2)
                    denom = small.tile([128, 2], f32, tag="denom")
                    recip = small.tile([128, 2], f32, tag="recip")
                    # denom = rowsum + 1  (the +1 is the normalized prefix)
                    nc.vector.tensor_scalar_add(denom[:, :], o_r[:, :, D], 1.0)
                    nc.vector.reciprocal(recip[:, :], denom[:, :])
                    recips.append(recip)
                return recips

            def emit_norms(g, o_tiles, recips):
                og_sb = ogp.tile([128, NREP * 128], f32, tag="og")
                def store_half(j):
                    for c in (2 * j, 2 * j + 1):
                        eng = nc.sync if c % 2 == 0 else nc.scalar
                        eng.dma_start(
                            out_d.ap()[g][:, c * 128 : (c + 1) * 128],
                            og_sb[:, c * 128 : (c + 1) * 128],
                        )
                for j in range(2):
                    o_r = o_tiles[j][:, :].rearrange("p (c x) -> p c x", c=2)
                    recip = recips[j]
                    if bcast_norm and j == 0:
                        nc.vector.tensor_tensor(
                            og_sb[:, j * 256 : (j + 1) * 256].rearrange(
                                "p (c x) -> p c x", c=2
                            ),
                            o_r[:, :, 0:D],
                            recip[:, :].broadcast_to([128, 2, D]),
                            ALU.mult,
                        )
                    else:
                        # normalize on the scalar engine (Copy shares the
                        # Exp table slot, no reload)
                        for i in range(2):
                            r = 2 * j + i
                            nc.scalar.activation(
                                og_sb[:, r * 128 : (r + 1) * 128],
                                o_r[:, i, 0:D],
                                AF.Copy,
                                scale=recip[:, i : i + 1],
                            )
                    if g >= 7:
                        # last group: store each half right after its
                        # normalize so the final bytes land sooner
                        store_half(j)
                if g < 7:
                    nc.sync.dma_start(out_d.ap()[g], og_sb[:, :])

            # software pipeline: S runs 3 groups ahead; next group's
            # stt/exp is issued before this group's norm copies so the
            # scalar queue never blocks the exp chain
            sT = {0: emit_s(0), 1: emit_s(1)}
            pT = {0: emit_softmax(0, sT.pop(0))}
            sT[2] = emit_s(2)
            prev = None  # (g, o_tiles, recips)
            for g in range(KVH):
                o_tiles = emit_av(g, pT.pop(g))
                recips = emit_denoms(g, o_tiles)
                if g + 1 < KVH:
                    pT[g + 1] = emit_softmax(g + 1, sT.pop(g + 1))
                if g + 3 < KVH:
                    sT[g + 3] = emit_s(g + 3)
                if prev is not None:
                    emit_norms(*prev)
                prev = (g, o_tiles, recips)
            emit_norms(*prev)

    nc.compile()
    return nc


def _get_nc(**kw):
    key = tuple(sorted(kw.items()))
    if key not in _BUILT:
        _BUILT[key] = _build_nc(**kw)
    return _BUILT[key]


def _reference_fallback(q, k, v, start_pos, mask, cache_k, cache_v):
    b, s, _ = q.shape
    start_pos = int(start_pos)
    t = start_pos + s
    xq = q.reshape(b, s, H, D).astype(np.float32)
    xk = k.reshape(b, s, KVH, D).astype(np.float32)
    xv = v.reshape(b, s, KVH, D).astype(np.float32)
    ck = np.array(cache_k[:b, :t], dtype=np.float32, copy=True)
    cv = np.array(cache_v[:b, :t], dtype=np.float32, copy=True)
    ck[:, start_pos:t] = xk
    cv[:, start_pos:t] = xv
    xqg = xq.reshape(b, s, KVH, NREP, D)
    scores = np.einsum("bsgrd,btgd->bgrst", xqg, ck) * SCALE
    scores = scores + np.asarray(mask, dtype=np.float32)[:, :, None]
    scores -= scores.max(axis=-1, keepdims=True)
    p = np.exp(scores)
    p /= p.sum(axis=-1, keepdims=True)
    out = np.einsum("bgrst,btgd->bsgrd", p, cv)
    return out.reshape(b, s, H * D).astype(np.float32)


def kernel(q, k, v, start_pos, freqs_cis, mask, cache_k, cache_v):
    q = np.asarray(q, dtype=np.float32)
    k = np.asarray(k, dtype=np.float32)
    v = np.asarray(v, dtype=np.float32)
    mask = np.asarray(mask, dtype=np.float32)
    sp = int(start_pos)

    fast_ok = (
        sp == START
        and q.shape == (B, S, DIM)
        and k.shape == (B, S, KV_DIM)
        and v.shape == (B, S, KV_DIM)
        and mask.shape == (1, 1, S, T)
        and not np.asarray(cache_k)[:B, :START].any()
        and not np.asarray(cache_v)[:B, :START].any()
    )
    if not fast_ok:
        return _reference_fallback(q, k, v, sp, mask, cache_k, cache_v)

    from concourse.bass_utils import run_bass_kernel_spmd

    nc = _get_nc(fp32r_s=FP32R_S, bcast_norm=BCAST_NORM, fp16_av=FP16_AV)

    m2d = mask[0, 0]  # [S, T]
    presum = np.exp(m2d[:, :START]).sum(axis=1)  # [S]
    mlive_t = m2d[:, START:].T - np.log(presum)[None, :]  # [t', s]
    mask_t4 = np.ascontiguousarray(np.tile(mlive_t, (1, NREP)), np.float32)

    # host layout prep (pure permutation): group-blocked transposes,
    # ones column for V
    # qT[b, g, d, r*S+s] = q[b, s, (g*NREP+r)*D + d], as fp16 hi|lo pair
    qt32 = q.reshape(B, S, KVH, NREP, D).transpose(0, 2, 4, 3, 1).reshape(
        B, KVH, 128, NREP * S
    )
    qt = np.ascontiguousarray(qt32, np.float32)
    # group 0 additionally as 4 contiguous 64KB chunks
    qt0 = np.ascontiguousarray(
        qt[:, 0].reshape(B, 128, 4, S).transpose(0, 2, 1, 3)
    )
    qt = np.ascontiguousarray(qt[:, 1:])
    # kT[b, g, d, t'] = k[b, t', g*D + d]
    kt = np.ascontiguousarray(
        k.reshape(B, S, KVH, D).transpose(0, 2, 3, 1), np.float32
    )
    v_dt = np.float16 if FP16_AV else np.float32
    vones = np.empty((B, S, KVH, D + 1), dtype=v_dt)
    vones[..., :D] = v.reshape(B, S, KVH, D)
    vones[..., D] = 1.0
    vones = np.ascontiguousarray(
        vones.reshape(B, S, 2, 4 * (D + 1)).transpose(0, 2, 1, 3)
    )

    in_maps = [
        {
            "qT0": qt0[b],
            "qT": qt[b],
            "kT": kt[b],
            "vones": vones[b],
            "maskT4": mask_t4,
        }
        for b in range(B)
    ]
    res = run_bass_kernel_spmd(nc, in_maps, list(range(NCORES)))
    # device out is [KVH, S, NREP*D] blocks; un-permute to [S, H*D]
    out = np.stack(
        [
            res.results[b]["out"].transpose(1, 0, 2).reshape(S, DIM)
            for b in range(B)
        ],
        axis=0,
    )
    return out
